# revision 1
# baseline (speedup 1.0000x reference)
"""Bilateral filter (7x7, dilation 1) Trainium2 Bass kernel.

Problem: input [2, 18, 1024, 1024] f32.
  filterable = input[:, :8]; params = -(input[:, 8:]**2)
  range coeffs = params[:, :8], sx = params[:, 8], sy = params[:, 9]
  out[c] = sum_taps w * f_c(shifted) / sum_taps w, c < 3
  w = exp(sum_c r_c (fn_c - f_c)^2 + sx dx^2 + sy dy^2), OOB taps masked.

Sharding: data-parallel over (batch, H): 8 cores, each gets 256 rows of one
batch image (+3 halo rows each side, sentinel-padded host-side).  Out-of-image
taps get weight exactly 0 because the sentinel (1e18) drives the quadratic
form to -huge and exp underflows to +0.

Per-core layout: H rows on partitions (128 x 2 blocks), W in chunks of 256 on
the free axis with the 8 filterable channels interleaved (x*8+c).  Row shifts
(oy) come from 7 row-shifted tile copies; column shifts (ox) are free-axis
offsets into the 6-column halo.

Engine split per tap: DVE sub/reduce/adds, ACT square/exp, GPSIMD r*d^2.
"""

import sys

if "/opt/trn_rl_repo" not in sys.path:
    sys.path.insert(0, "/opt/trn_rl_repo")

import numpy as np

import concourse.bass as bass
import concourse.mybir as mybir
from concourse.bacc import Bacc
from concourse.tile import TileContext

FP32 = mybir.dt.float32

B, C_ALL, H, W = 2, 18, 1024, 1024
CF = 8                      # filterable channels
CO = 3                      # output channels
KS, RAD = 7, 3
HC = H * B // 8             # 256 output rows per core
HIN = HC + 2 * RAD          # 262 input rows per core (halo padded host-side)
WC = 256                    # W chunk
NW = W // WC                # 4
NHB = HC // 128             # 2
SENT = 1.0e18               # sentinel padding value -> tap weight exp(-huge)=0
D2 = [9.0, 4.0, 1.0, 0.0, 1.0, 4.0, 9.0]   # (k-3)^2 for k in 0..6
D2IDX = [3, 2, 1, 0, 1, 2, 3]              # index into [0,1,4,9]
D2VALS = [0.0, 1.0, 4.0, 9.0]

_CACHED = {}
TAP_SET = None   # optional [(i,j)] subset for debugging


def _ilv(ap, n, c=CF):
    """View flat [128, n*c] region as [128, n, c] (channel-interleaved)."""
    return ap.rearrange("p (x c) -> p x c", c=c)


def build_nc(macros=None):
    nc = Bacc()
    x = nc.dram_tensor("x", [C_ALL, HIN, W], FP32, kind="ExternalInput")
    y = nc.dram_tensor("y", [CO, HC, W], FP32, kind="ExternalOutput")

    if macros is None:
        macros = [(hb, wck) for hb in range(NHB) for wck in range(NW)]
    with TileContext(nc) as tc:
        with (
            tc.tile_pool(name="fpool", bufs=1) as fpool,
            tc.tile_pool(name="cpool", bufs=1) as cpool,
            tc.tile_pool(name="dpool", bufs=5) as dpool,
            tc.tile_pool(name="spool", bufs=4) as spool,
            tc.tile_pool(name="ppool", bufs=1, space="PSUM") as ppool,
        ):
            for hb, wcki in macros:
                _macro(nc, tc, x, y, fpool, cpool, dpool, spool, ppool, hb, wcki)
    nc.compile()
    return nc


def _macro(nc, tc, x, y, fpool, cpool, dpool, spool, ppool, hb, wck):
    w0 = wck * WC
    r0 = hb * 128
    wtile = WC + 2 * RAD
    # tile col t  <->  image col w0 - 3 + t
    lo = RAD if wck == 0 else 0
    hi = wtile - RAD if wck == NW - 1 else wtile

    # ---- load + interleave the 7 row-shifted filterable tile sets ----
    F = []
    for oy in range(KS):
        Fi = fpool.tile([128, wtile * CF], FP32, tag=f"F{oy}", bufs=1,
                        name=f"F{oy}_{hb}_{wck}")
        for c in range(CF):
            pl = fpool.tile([128, wtile], FP32, tag="pl", bufs=3,
                            name=f"pl_{hb}_{wck}_{oy}_{c}")
            if lo > 0:
                nc.gpsimd.memset(pl[:, 0:lo], SENT)
            if hi < wtile:
                nc.gpsimd.memset(pl[:, hi:wtile], SENT)
            nc.sync.dma_start(
                out=pl[:, lo:hi],
                in_=x[c, r0 + oy : r0 + oy + 128, w0 - RAD + lo : w0 - RAD + hi],
            )
            # interleave: Fi[p, t*8+c] = pl[p, t]   (ACT, strided out)
            nc.scalar.copy(_ilv(Fi[:], wtile)[:, :, c], pl[:])
        F.append(Fi)
    Fc = _ilv(F[RAD][:, RAD * CF : (RAD + WC) * CF], WC)      # center view

    # ---- params: R (interleaved), sx2, sy2 ----
    R = cpool.tile([128, WC * CF], FP32, tag="R", name=f"R_{hb}_{wck}")
    for c in range(CF):
        pp = fpool.tile([128, WC], FP32, tag="pp", bufs=2,
                        name=f"pp_{hb}_{wck}_{c}")
        nc.sync.dma_start(
            out=pp[:], in_=x[CF + c, r0 + RAD : r0 + RAD + 128, w0 : w0 + WC])
        nc.vector.scalar_tensor_tensor(
            _ilv(R[:], WC)[:, :, c], pp[:], -1.0, pp[:],
            mybir.AluOpType.mult, mybir.AluOpType.mult)
    sxy2 = cpool.tile([128, 2 * WC], FP32, tag="sxy2", name=f"sxy2_{hb}_{wck}")
    for k in range(2):
        pp = fpool.tile([128, WC], FP32, tag="pp", bufs=2,
                        name=f"pps_{hb}_{wck}_{k}")
        nc.sync.dma_start(
            out=pp[:], in_=x[2 * CF + k, r0 + RAD : r0 + RAD + 128, w0 : w0 + WC])
        nc.vector.scalar_tensor_tensor(
            sxy2[:, k * WC : (k + 1) * WC], pp[:], -1.0, pp[:],
            mybir.AluOpType.mult, mybir.AluOpType.mult)
    sx2 = sxy2[:, 0:WC]
    sy2 = sxy2[:, WC : 2 * WC]

    # ---- Asp[a][b] = a*sx2 + b*sy2  (spatial log-weight, 16 combos) ----
    Ab = spool.tile([128, 4 * WC], FP32, tag="Ab", bufs=2, name=f"Ab_{hb}_{wck}")
    for bi, bval in enumerate(D2VALS):
        nc.vector.tensor_scalar_mul(
            Ab[:, bi * WC : (bi + 1) * WC], sy2, float(bval))
    Asp = cpool.tile([128, 16 * WC], FP32, tag="Asp", name=f"Asp_{hb}_{wck}")
    for ai, aval in enumerate(D2VALS):
        for bi in range(4):
            nc.vector.scalar_tensor_tensor(
                Asp[:, (ai * 4 + bi) * WC : (ai * 4 + bi + 1) * WC],
                sx2, float(aval), Ab[:, bi * WC : (bi + 1) * WC],
                mybir.AluOpType.mult, mybir.AluOpType.add)

    # ---- accumulators ----
    acc = cpool.tile([128, WC * CO], FP32, tag="acc", name=f"acc_{hb}_{wck}")
    wsum = cpool.tile([128, WC], FP32, tag="wsum", name=f"wsum_{hb}_{wck}")
    nc.gpsimd.memset(acc[:], 0.0)
    nc.gpsimd.memset(wsum[:], 0.0)

    # ---- 49 taps ----
    taps = TAP_SET if TAP_SET is not None else [(i, j) for i in range(KS) for j in range(KS)]
    for i, j in taps:            # oy = i - 3, ox = j - 3
        if True:
            Fi = F[i]
            sh = _ilv(Fi[:, j * CF : (j + WC) * CF], WC)     # shifted read
            d = dpool.tile([128, WC * CF], FP32, tag="d",
                           name=f"d_{hb}_{wck}_{i}_{j}")
            nc.vector.tensor_sub(_ilv(d[:], WC), sh, Fc)
            nc.scalar.activation(d[:], d[:], mybir.ActivationFunctionType.Square)
            nc.gpsimd.tensor_mul(d[:], R[:], d[:])
            s = spool.tile([128, WC], FP32, tag="s",
                           name=f"s_{hb}_{wck}_{i}_{j}")
            nc.vector.tensor_reduce(s[:], _ilv(d[:], WC),
                                    axis=mybir.AxisListType.X,
                                    op=mybir.AluOpType.add)
            k = (D2IDX[j] * 4 + D2IDX[i]) * WC
            nc.vector.tensor_add(s[:], s[:], Asp[:, k : k + WC])
            w_t = spool.tile([128, WC], FP32, tag="w",
                             name=f"w_{hb}_{wck}_{i}_{j}")
            nc.scalar.activation(w_t[:], s[:], mybir.ActivationFunctionType.Exp)
            nc.vector.tensor_add(wsum[:], wsum[:], w_t[:])
            t3 = spool.tile([128, WC * CO], FP32, tag="t3",
                            name=f"t3_{hb}_{wck}_{i}_{j}")
            w_b = w_t[:].unsqueeze(2).broadcast_to([128, WC, CO])
            f3 = _ilv(Fi[:, j * CF : (j + WC) * CF], WC)[:, :, 0:CO]
            nc.vector.tensor_mul(_ilv(t3[:], WC, CO), w_b, f3)
            nc.vector.tensor_add(acc[:], acc[:], t3[:])

    # ---- out = acc / wsum ----
    rec = spool.tile([128, WC], FP32, tag="s", name=f"rec_{hb}_{wck}")
    nc.vector.reciprocal(rec[:], wsum[:])
    out3 = spool.tile([128, WC * CO], FP32, tag="t3", name=f"out3_{hb}_{wck}")
    rec_b = rec[:].unsqueeze(2).broadcast_to([128, WC, CO])
    nc.vector.tensor_mul(_ilv(out3[:], WC, CO), rec_b, _ilv(acc[:], WC, CO))
    for c in range(CO):
        oc = spool.tile([128, WC], FP32, tag="oc", name=f"oc_{hb}_{wck}_{c}")
        nc.scalar.copy(oc[:], _ilv(out3[:], WC, CO)[:, :, c])
        nc.sync.dma_start(out=y[c, r0 : r0 + 128, w0 : w0 + WC], in_=oc[:])


def shard_inputs(input):
    """input [2,18,1024,1024] -> 8 per-core slabs [18, 262, 1024]."""
    input = np.asarray(input, dtype=np.float32)
    per_b = 4
    rows = H // per_b
    in_maps = []
    for core in range(8):
        b, q = divmod(core, per_b)
        r0 = q * rows
        slab = np.full((C_ALL, HIN, W), SENT, dtype=np.float32)
        s_lo = max(r0 - RAD, 0)
        s_hi = min(r0 + rows + RAD, H)
        slab[:, s_lo - (r0 - RAD) : s_hi - (r0 - RAD), :] = input[b, :, s_lo:s_hi, :]
        in_maps.append({"x": np.ascontiguousarray(slab)})
    return in_maps


def assemble(results):
    out = np.empty((B, CO, H, W), dtype=np.float32)
    rows = H // 4
    for core in range(8):
        b, q = divmod(core, 4)
        out[b, :, q * rows : (q + 1) * rows, :] = results[core]["y"]
    return out


def kernel(input):
    from concourse.bass_utils import run_bass_kernel_spmd

    if "nc" not in _CACHED:
        _CACHED["nc"] = build_nc()
    in_maps = shard_inputs(input)
    res = run_bass_kernel_spmd(_CACHED["nc"], in_maps, list(range(8)))
    return assemble(res.results)



# revision 14
# speedup vs baseline: 2.3128x; 2.3128x over previous
"""Bilateral filter (7x7, dilation 1) Trainium2 Bass kernel, v3.

Problem: input [2, 18, 1024, 1024] f32.
  filterable f = input[:, :8]; params p = input[:, 8:]
  logw(tap) = -sum_c p_c^2 (fn_c - f_c)^2 - p8^2 dx^2 - p9^2 dy^2
  out_c = sum_taps exp(logw) fn_c / sum_taps exp(logw),  c < 3

Sharding: 8 cores x (one batch quarter of 256 rows + 3 halo rows), as in
the baseline.  Out-of-image taps are killed by a 1e18 sentinel pad (the
quadratic form then underflows exp to +0).

Per-core layout (bf16):  macro = 64 output rows x full width.
  tiles put (channel-pair c in {0,1}, row r in 0..63) on partitions and
  (segment g in 0..3, x) on the free axis, so channel ci = 2g + c.
  Row shifts -> 7 DMA'd tile variants F[oy]; column shifts -> free-axis
  offsets (DVE needs in0/in1 offsets with equal parity, so the center
  operand comes from F[3] (odd offset 3) for odd j and from a separate
  even-phase tile Fce (offset 2) for even j).

Per tap: d = Fsh - Fc (DVE), d2 = Square(d) (ACT), q = (p^2)*d2 (DVE or
GPSIMD, flat), then PE matmuls with a -1 row-selector lhsT contract the
channels of q into PSUM (2 taps per bank, M=64 at partitions 0/64), plus
one matmul with lhsT = -(dx^2, dy^2) selector against the (p8^2, p9^2)
tile for the spatial term.  w = Exp(PSUM) on ACT.  acc_c and wsum are
accumulated over all taps by PE matmuls with a +1 selector against
WF_c = w * G_c (gathered neighbor tiles) and w itself.
"""

import sys

if "/opt/trn_rl_repo" not in sys.path:
    sys.path.insert(0, "/opt/trn_rl_repo")

import numpy as np
import ml_dtypes

import concourse.bass as bass
import concourse.mybir as mybir
from concourse.bacc import Bacc
from concourse.tile import TileContext

FP32 = mybir.dt.float32
BF16 = mybir.dt.bfloat16
ALU = mybir.AluOpType
ACTF = mybir.ActivationFunctionType

B, C_ALL, H, W = 2, 18, 1024, 1024
CF, CO = 8, 3
KS, RAD = 7, 3
HC = 256                      # output rows per core
HIN = HC + 2 * RAD            # slab rows (262)
WP = W + 2 * RAD              # padded slab width (1030)
MR = 64                       # macro rows
NM = HC // MR                 # 4 macros
SEG = 4                       # channel-pair segments per tile
SW = W + 8                    # segment width in F tiles (1032, even)
SENT = 1.0e18
BF = ml_dtypes.bfloat16

_CACHED = {}

# tap pairing: 48 non-center taps in 24 pairs + (center, dead)
_TAPS = [(i, j) for i in range(KS) for j in range(KS) if not (i == RAD and j == RAD)]
PAIRS = [(_TAPS[2 * k], _TAPS[2 * k + 1]) for k in range(24)]
CENTER_PAIR_IDX = 24          # slot0 = center tap (w=1), slot1 = dead (w=0)

N_CONST = 19                  # SELN, SELP, 16 spatial matrices, SHIFTC


def _seg(ap, width=SW):
    return ap.rearrange("p (s x) -> p s x", s=SEG)


def build_consts():
    """[18, 128, 64] bf16: row-selector and spatial lhsT matrices."""
    c = np.zeros((N_CONST, 128, 64), dtype=np.float32)
    r = np.arange(64)
    for g in range(2):
        c[0, g * 64 + r, r] = -1.0          # SELN
        c[1, g * 64 + r, r] = 1.0           # SELP
    for a in range(4):                       # dx2 index
        for b in range(4):                   # dy2 index
            m = 2 + a * 4 + b
            c[m, 0 * 64 + r, r] = -float([0, 1, 4, 9][a])
            c[m, 1 * 64 + r, r] = -float([0, 1, 4, 9][b])
    c[18, 64 + r, r] = 1.0                   # SHIFTC: out[m] = in[m + 64]
    return c.astype(BF)


D2IDX = [3, 2, 1, 0, 1, 2, 3]


def build_nc():
    nc = Bacc()
    fil = nc.dram_tensor("fil", [CF, HIN, WP], BF16, kind="ExternalInput")
    prm = nc.dram_tensor("prm", [10, HC, W], BF16, kind="ExternalInput")
    cst = nc.dram_tensor("cst", [N_CONST, 128, 64], BF16, kind="ExternalInput")
    y = nc.dram_tensor("y", [CO, HC, W], FP32, kind="ExternalOutput")

    with TileContext(nc) as tc:
        with (
            tc.tile_pool(name="const", bufs=1) as kpool,
            tc.tile_pool(name="fpool", bufs=1) as fpool,
            tc.tile_pool(name="dpool", bufs=3) as dpool,
            tc.tile_pool(name="wpool", bufs=2) as wpool,
            tc.tile_pool(name="opool", bufs=2) as opool,
            tc.tile_pool(name="ppool", bufs=4, space="PSUM") as ppool,
        ):
            CST = kpool.tile([128, N_CONST * 64], BF16, name="CST")
            for m in range(N_CONST):
                nc.sync.dma_start(out=CST[:, m * 64:(m + 1) * 64], in_=cst[m])
            SELN = CST[:, 0:64]
            SELP = CST[:, 64:128]
            SHIFTC = CST[:, 18 * 64:19 * 64]

            def SPAT(i, j):
                m = 2 + D2IDX[j] * 4 + D2IDX[i]
                return CST[:, m * 64:(m + 1) * 64]

            for mi in range(NM):
                _macro(nc, tc, fil, prm, y, kpool, fpool, dpool, wpool, opool,
                       ppool, mi, SELN, SELP, SPAT, SHIFTC)
    nc.compile()
    return nc


def _macro(nc, tc, fil, prm, y, kpool, fpool, dpool, wpool, opool, ppool, mi,
           SELN, SELP, SPAT, SHIFTC):
    r0 = mi * MR

    # ---- F tiles: 7 row-shift variants + even-phase center ----
    F = []
    for oy in range(KS):
        Ft = fpool.tile([128, SEG * SW], BF16, tag=f"F{oy}", name=f"F{oy}_{mi}")
        for g in range(SEG):
            for c in range(2):
                nc.sync.dma_start(
                    out=_seg(Ft[:])[c * 64:(c + 1) * 64, g, 0:WP],
                    in_=fil[2 * g + c, r0 + oy:r0 + oy + MR, 0:WP])
        F.append(Ft)
    Fce = fpool.tile([128, SEG * SW], BF16, tag="Fce", name=f"Fce_{mi}")
    for g in range(SEG):
        for c in range(2):
            nc.sync.dma_start(
                out=_seg(Fce[:])[c * 64:(c + 1) * 64, g, 0:WP - 1],
                in_=fil[2 * g + c, r0 + RAD:r0 + RAD + MR, 1:WP])
    # center views at offset 3 (odd phase, in F[3]) and offset 2 (even, Fce)
    Fc_o = _seg(F[RAD][:])[:, :, 3:3 + W]
    Fc_e = _seg(Fce[:])[:, :, 2:2 + W]

    # ---- R = p_c^2 (bf16), SXY = (p8^2, p9^2) ----
    Rw = fpool.tile([128, SEG * W], BF16, tag="R", name=f"R_{mi}")
    for g in range(SEG):
        for c in range(2):
            nc.sync.dma_start(
                out=Rw[c * 64:(c + 1) * 64, g * W:(g + 1) * W],
                in_=prm[2 * g + c, r0:r0 + MR, :])
    nc.vector.tensor_mul(Rw[:], Rw[:], Rw[:])
    SXY = fpool.tile([128, W], BF16, tag="SXY", name=f"SXY_{mi}")
    for k in range(2):
        nc.sync.dma_start(out=SXY[k * 64:(k + 1) * 64, :],
                          in_=prm[8 + k, r0:r0 + MR, :])
    nc.vector.tensor_mul(SXY[:], SXY[:], SXY[:])

    # ---- PSUM accumulators: acc0,acc1 | acc2,wsum  (x2 N-halves) ----
    ACC = [ppool.tile([128, 512], FP32, tag="ACC", name=f"ACC{k}_{mi}")
           for k in range(4)]   # [A_h0, A_h1, B_h0, B_h1]

    nq = 0
    for pi, pair in enumerate(PAIRS + [((RAD, RAD), None)]):
        w_t = wpool.tile([128, W], BF16, tag="w", name=f"w_{mi}_{pi}", bufs=3)
        if pi == CENTER_PAIR_IDX:
            nc.gpsimd.memset(w_t[0:64, :], 1.0)
            nc.gpsimd.memset(w_t[64:128, :], 0.0)
            g_taps = (pair[0], pair[0])
        else:
            SP = [ppool.tile([128, 512], FP32, tag="SP", name=f"SP{h}_{mi}_{pi}",
                             bufs=4) for h in range(2)]
            for sl, (i, j) in enumerate(pair):
                Fc = Fc_o if (j % 2 == 1) else Fc_e
                d = dpool.tile([128, SEG * W], BF16, tag="d",
                               name=f"d_{mi}_{pi}_{sl}", bufs=3)
                dv = d[:].rearrange("p (s x) -> p s x", s=SEG)
                nc.vector.tensor_sub(dv, _seg(F[i][:])[:, :, j:j + W], Fc)
                d2 = dpool.tile([128, SEG * W], BF16, tag="d2",
                                name=f"d2_{mi}_{pi}_{sl}", bufs=2)
                nc.scalar.activation(d2[:], d[:], ACTF.Square)
                if nq % 2 == 0:
                    nc.gpsimd.tensor_mul(d2[:], Rw[:], d2[:])
                else:
                    nc.vector.tensor_mul(d2[:], Rw[:], d2[:])
                nq += 1
                pos = slice(sl * 64, sl * 64 + 64)
                for h in range(2):
                    for g in range(SEG):
                        nc.tensor.matmul(
                            SP[h][pos, :], SELN,
                            d2[:, g * W + h * 512:g * W + h * 512 + 512],
                            start=(g == 0), stop=False, skip_group_check=True)
                    nc.tensor.matmul(
                        SP[h][pos, :], SPAT(i, j),
                        SXY[:, h * 512:h * 512 + 512],
                        start=False, stop=True, skip_group_check=True)
            for h in range(2):
                nc.scalar.activation(w_t[:, h * 512:h * 512 + 512], SP[h][:],
                                     ACTF.Exp)
            g_taps = pair

        # ---- gather tiles + weighted accumulation ----
        first = pi == 0
        last = pi == CENTER_PAIR_IDX
        WFs = []
        for c in range(CO):
            G = wpool.tile([128, W], BF16, tag=f"G{c}", name=f"G{c}_{mi}_{pi}",
                           bufs=2)
            for sl in range(2):
                i, j = g_taps[sl]
                nc.sync.dma_start(
                    out=G[sl * 64:sl * 64 + 64, :],
                    in_=fil[c, r0 + i:r0 + i + MR, j:j + W])
            WF = wpool.tile([128, W], BF16, tag=f"WF{c}",
                            name=f"WF{c}_{mi}_{pi}", bufs=2)
            nc.vector.tensor_mul(WF[:], w_t[:], G[:])
            WFs.append(WF)
        for h in range(2):
            hs = slice(h * 512, h * 512 + 512)
            nc.tensor.matmul(ACC[h][0:64, :], SELP, WFs[0][:, hs],
                             start=first, stop=last, skip_group_check=True)
            nc.tensor.matmul(ACC[h][64:128, :], SELP, WFs[1][:, hs],
                             start=first, stop=last, skip_group_check=True)
            nc.tensor.matmul(ACC[2 + h][0:64, :], SELP, WFs[2][:, hs],
                             start=first, stop=last, skip_group_check=True)
            nc.tensor.matmul(ACC[2 + h][64:128, :], SELP, w_t[:, hs],
                             start=first, stop=last, skip_group_check=True)

    # ---- out = acc / wsum ----
    # wsum lives at partitions 64:128 of ACC[2+h]; DMA-replicate it to both
    # partition halves so every elementwise op sees matching base partitions.
    for h in range(2):
        hs = slice(h * 512, h * 512 + 512)
        AS = opool.tile([128, 512], BF16, tag="as", name=f"as_{mi}_{h}")
        nc.scalar.copy(AS[:], ACC[2 + h][:])
        TMP = ppool.tile([128, 512], FP32, tag="SP", name=f"wtmp_{mi}_{h}",
                         bufs=4)
        nc.tensor.matmul(TMP[0:64, :], SHIFTC, AS[:], start=True, stop=True,
                         skip_group_check=True)
        RW = opool.tile([128, 512], FP32, tag="rw", name=f"rw_{mi}_{h}")
        nc.vector.reciprocal(RW[0:64, :], TMP[0:64, :])
        nc.vector.reciprocal(RW[64:128, :], ACC[2 + h][64:128, :])
        o = opool.tile([128, 512], FP32, tag="o", name=f"o_{mi}_{h}")
        o2 = opool.tile([128, 512], FP32, tag="o2", name=f"o2_{mi}_{h}")
        nc.vector.tensor_mul(o[0:64, :], ACC[h][0:64, :], RW[0:64, :])
        nc.vector.tensor_mul(o[64:128, :], ACC[h][64:128, :], RW[64:128, :])
        nc.vector.tensor_mul(o2[0:64, :], ACC[2 + h][0:64, :], RW[0:64, :])
        nc.sync.dma_start(out=y[0, r0:r0 + MR, hs], in_=o[0:64, :])
        nc.sync.dma_start(out=y[1, r0:r0 + MR, hs], in_=o[64:128, :])
        nc.sync.dma_start(out=y[2, r0:r0 + MR, hs], in_=o2[0:64, :])


def shard_inputs(input):
    """input [2,18,1024,1024] f32 -> per-core {fil, prm, cst} bf16."""
    input = np.asarray(input, dtype=np.float32)
    cst = build_consts()
    fil_all = input[:, :CF]
    prm_all = input[:, CF:]
    in_maps = []
    for core in range(8):
        b, q = divmod(core, 4)
        r0 = q * HC
        slab = np.full((CF, HIN, WP), SENT, dtype=np.float32)
        s_lo = max(r0 - RAD, 0)
        s_hi = min(r0 + HC + RAD, H)
        slab[:, s_lo - (r0 - RAD):s_hi - (r0 - RAD), RAD:RAD + W] = \
            fil_all[b, :, s_lo:s_hi, :]
        prm = prm_all[b, :, r0:r0 + HC, :]
        in_maps.append({
            "fil": np.ascontiguousarray(slab.astype(BF)),
            "prm": np.ascontiguousarray(prm.astype(BF)),
            "cst": cst,
        })
    return in_maps


def assemble(results):
    out = np.empty((B, CO, H, W), dtype=np.float32)
    for core in range(8):
        b, q = divmod(core, 4)
        out[b, :, q * HC:(q + 1) * HC, :] = results[core]["y"]
    return out


def kernel(input):
    from concourse.bass_utils import run_bass_kernel_spmd

    if "nc" not in _CACHED:
        _CACHED["nc"] = build_nc()
    in_maps = shard_inputs(input)
    res = run_bass_kernel_spmd(_CACHED["nc"], in_maps, list(range(8)))
    return assemble(res.results)


# revision 15
# speedup vs baseline: 3.6062x; 1.5592x over previous
"""Bilateral filter (7x7, dilation 1) Trainium2 Bass kernel, v3.

Problem: input [2, 18, 1024, 1024] f32.
  filterable f = input[:, :8]; params p = input[:, 8:]
  logw(tap) = -sum_c p_c^2 (fn_c - f_c)^2 - p8^2 dx^2 - p9^2 dy^2
  out_c = sum_taps exp(logw) fn_c / sum_taps exp(logw),  c < 3

Sharding: 8 cores x (one batch quarter of 256 rows + 3 halo rows), as in
the baseline.  Out-of-image taps are killed by a 1e18 sentinel pad (the
quadratic form then underflows exp to +0).

Per-core layout (bf16):  macro = 64 output rows x full width.
  tiles put (channel-pair c in {0,1}, row r in 0..63) on partitions and
  (segment g in 0..3, x) on the free axis, so channel ci = 2g + c.
  Row shifts -> 7 DMA'd tile variants F[oy]; column shifts -> free-axis
  offsets (DVE needs in0/in1 offsets with equal parity, so the center
  operand comes from F[3] (odd offset 3) for odd j and from a separate
  even-phase tile Fce (offset 2) for even j).

Per tap: d = Fsh - Fc (DVE), d2 = Square(d) (ACT), q = (p^2)*d2 (DVE or
GPSIMD, flat), then PE matmuls with a -1 row-selector lhsT contract the
channels of q into PSUM (2 taps per bank, M=64 at partitions 0/64), plus
one matmul with lhsT = -(dx^2, dy^2) selector against the (p8^2, p9^2)
tile for the spatial term.  w = Exp(PSUM) on ACT.  acc_c and wsum are
accumulated over all taps by PE matmuls with a +1 selector against
WF_c = w * G_c (gathered neighbor tiles) and w itself.
"""

import sys

if "/opt/trn_rl_repo" not in sys.path:
    sys.path.insert(0, "/opt/trn_rl_repo")

import numpy as np
import ml_dtypes

import concourse.bass as bass
import concourse.mybir as mybir
from concourse.bacc import Bacc
from concourse.tile import TileContext

FP32 = mybir.dt.float32
BF16 = mybir.dt.bfloat16
ALU = mybir.AluOpType
ACTF = mybir.ActivationFunctionType

B, C_ALL, H, W = 2, 18, 1024, 1024
CF, CO = 8, 3
KS, RAD = 7, 3
HC = 256                      # output rows per core
HIN = HC + 2 * RAD            # slab rows (262)
WP = W + 2 * RAD              # padded slab width (1030)
MR = 64                       # macro rows
NM = HC // MR                 # 4 macros
SEG = 4                       # channel-pair segments per tile
SW = W + 8                    # segment width in F tiles (1032, even)
SENT = 1.0e18
BF = ml_dtypes.bfloat16

_CACHED = {}

# tap pairing: 48 non-center taps in 24 pairs + (center, dead)
_TAPS = [(i, j) for i in range(KS) for j in range(KS) if not (i == RAD and j == RAD)]
PAIRS = [(_TAPS[2 * k], _TAPS[2 * k + 1]) for k in range(24)]
CENTER_PAIR_IDX = 24          # slot0 = center tap (w=1), slot1 = dead (w=0)

N_CONST = 19                  # SELN, SELP, 16 spatial matrices, SHIFTC


def _seg(ap, width=SW):
    return ap.rearrange("p (s x) -> p s x", s=SEG)


def build_consts():
    """[18, 128, 64] bf16: row-selector and spatial lhsT matrices."""
    c = np.zeros((N_CONST, 128, 64), dtype=np.float32)
    r = np.arange(64)
    for g in range(2):
        c[0, g * 64 + r, r] = -1.0          # SELN
        c[1, g * 64 + r, r] = 1.0           # SELP
    for a in range(4):                       # dx2 index
        for b in range(4):                   # dy2 index
            m = 2 + a * 4 + b
            c[m, 0 * 64 + r, r] = -float([0, 1, 4, 9][a])
            c[m, 1 * 64 + r, r] = -float([0, 1, 4, 9][b])
    c[18, 64 + r, r] = 1.0                   # SHIFTC: out[m] = in[m + 64]
    return c.astype(BF)


D2IDX = [3, 2, 1, 0, 1, 2, 3]


def build_nc():
    nc = Bacc()
    fil = nc.dram_tensor("fil", [CF, HIN, WP], BF16, kind="ExternalInput")
    prm = nc.dram_tensor("prm", [10, HC, W], BF16, kind="ExternalInput")
    cst = nc.dram_tensor("cst", [N_CONST, 128, 64], BF16, kind="ExternalInput")
    y = nc.dram_tensor("y", [CO, HC, W], FP32, kind="ExternalOutput")

    with TileContext(nc) as tc:
        with (
            tc.tile_pool(name="const", bufs=1) as kpool,
            tc.tile_pool(name="fpool", bufs=1) as fpool,
            tc.tile_pool(name="dpool", bufs=3) as dpool,
            tc.tile_pool(name="wpool", bufs=2) as wpool,
            tc.tile_pool(name="opool", bufs=2) as opool,
            tc.tile_pool(name="ppool", bufs=4, space="PSUM") as ppool,
        ):
            CST = kpool.tile([128, N_CONST * 64], BF16, name="CST")
            for m in range(N_CONST):
                nc.sync.dma_start(out=CST[:, m * 64:(m + 1) * 64], in_=cst[m])
            SELN = CST[:, 0:64]
            SELP = CST[:, 64:128]
            SHIFTC = CST[:, 18 * 64:19 * 64]

            def SPAT(i, j):
                m = 2 + D2IDX[j] * 4 + D2IDX[i]
                return CST[:, m * 64:(m + 1) * 64]

            for mi in range(NM):
                _macro(nc, tc, fil, prm, y, kpool, fpool, dpool, wpool, opool,
                       ppool, mi, SELN, SELP, SPAT, SHIFTC)
    nc.compile()
    return nc


def _macro(nc, tc, fil, prm, y, kpool, fpool, dpool, wpool, opool, ppool, mi,
           SELN, SELP, SPAT, SHIFTC):
    r0 = mi * MR

    # ---- F tiles: 7 row-shift variants + even-phase center ----
    F = []
    for oy in range(KS):
        Ft = fpool.tile([128, SEG * SW], BF16, tag=f"F{oy}", name=f"F{oy}_{mi}")
        for g in range(SEG):
            for c in range(2):
                nc.sync.dma_start(
                    out=_seg(Ft[:])[c * 64:(c + 1) * 64, g, 0:WP],
                    in_=fil[2 * g + c, r0 + oy:r0 + oy + MR, 0:WP])
        F.append(Ft)
    Fce = fpool.tile([128, SEG * SW], BF16, tag="Fce", name=f"Fce_{mi}")
    for g in range(SEG):
        for c in range(2):
            nc.sync.dma_start(
                out=_seg(Fce[:])[c * 64:(c + 1) * 64, g, 0:WP - 1],
                in_=fil[2 * g + c, r0 + RAD:r0 + RAD + MR, 1:WP])
    # center views at offset 3 (odd phase, in F[3]) and offset 2 (even, Fce)
    Fc_o = _seg(F[RAD][:])[:, :, 3:3 + W]
    Fc_e = _seg(Fce[:])[:, :, 2:2 + W]

    # ---- R = p_c^2 (bf16), SXY = (p8^2, p9^2) ----
    Rw = fpool.tile([128, SEG * W], BF16, tag="R", name=f"R_{mi}")
    for g in range(SEG):
        for c in range(2):
            nc.sync.dma_start(
                out=Rw[c * 64:(c + 1) * 64, g * W:(g + 1) * W],
                in_=prm[2 * g + c, r0:r0 + MR, :])
    nc.vector.tensor_mul(Rw[:], Rw[:], Rw[:])
    SXY = fpool.tile([128, W], BF16, tag="SXY", name=f"SXY_{mi}")
    for k in range(2):
        nc.sync.dma_start(out=SXY[k * 64:(k + 1) * 64, :],
                          in_=prm[8 + k, r0:r0 + MR, :])
    nc.vector.tensor_mul(SXY[:], SXY[:], SXY[:])

    # ---- PSUM accumulators: acc0,acc1 | acc2,wsum  (x2 N-halves) ----
    ACC = [ppool.tile([128, 512], FP32, tag="ACC", name=f"ACC{k}_{mi}")
           for k in range(4)]   # [A_h0, A_h1, B_h0, B_h1]

    nq = 0
    for pi, pair in enumerate(PAIRS + [((RAD, RAD), None)]):
        w_t = wpool.tile([128, W], BF16, tag="w", name=f"w_{mi}_{pi}", bufs=3)
        if pi == CENTER_PAIR_IDX:
            nc.gpsimd.memset(w_t[0:64, :], 1.0)
            nc.gpsimd.memset(w_t[64:128, :], 0.0)
            g_taps = (pair[0], pair[0])
        else:
            SP = [ppool.tile([128, 512], FP32, tag="SP", name=f"SP{h}_{mi}_{pi}",
                             bufs=4) for h in range(2)]
            for sl, (i, j) in enumerate(pair):
                Fc = Fc_o if (j % 2 == 1) else Fc_e
                d = dpool.tile([128, SEG * W], BF16, tag="d",
                               name=f"d_{mi}_{pi}_{sl}", bufs=3)
                dv = d[:].rearrange("p (s x) -> p s x", s=SEG)
                nc.vector.tensor_sub(dv, _seg(F[i][:])[:, :, j:j + W], Fc)
                d2 = dpool.tile([128, SEG * W], BF16, tag="d2",
                                name=f"d2_{mi}_{pi}_{sl}", bufs=2)
                nc.scalar.activation(d2[:], d[:], ACTF.Square)
                nc.vector.tensor_mul(d2[:], Rw[:], d2[:])
                nq += 1
                pos = slice(sl * 64, sl * 64 + 64)
                for h in range(2):
                    for g in range(SEG):
                        nc.tensor.matmul(
                            SP[h][pos, :], SELN,
                            d2[:, g * W + h * 512:g * W + h * 512 + 512],
                            start=(g == 0), stop=False, skip_group_check=True)
                    nc.tensor.matmul(
                        SP[h][pos, :], SPAT(i, j),
                        SXY[:, h * 512:h * 512 + 512],
                        start=False, stop=True, skip_group_check=True)
            for h in range(2):
                nc.scalar.activation(w_t[:, h * 512:h * 512 + 512], SP[h][:],
                                     ACTF.Exp)
            g_taps = pair

        # ---- gather tiles + weighted accumulation ----
        first = pi == 0
        last = pi == CENTER_PAIR_IDX
        WFs = []
        for c in range(CO):
            G = wpool.tile([128, W], BF16, tag=f"G{c}", name=f"G{c}_{mi}_{pi}",
                           bufs=2)
            for sl in range(2):
                i, j = g_taps[sl]
                nc.sync.dma_start(
                    out=G[sl * 64:sl * 64 + 64, :],
                    in_=fil[c, r0 + i:r0 + i + MR, j:j + W])
            WF = wpool.tile([128, W], BF16, tag=f"WF{c}",
                            name=f"WF{c}_{mi}_{pi}", bufs=2)
            nc.vector.tensor_mul(WF[:], w_t[:], G[:])
            WFs.append(WF)
        for h in range(2):
            hs = slice(h * 512, h * 512 + 512)
            nc.tensor.matmul(ACC[h][0:64, :], SELP, WFs[0][:, hs],
                             start=first, stop=last, skip_group_check=True)
            nc.tensor.matmul(ACC[h][64:128, :], SELP, WFs[1][:, hs],
                             start=first, stop=last, skip_group_check=True)
            nc.tensor.matmul(ACC[2 + h][0:64, :], SELP, WFs[2][:, hs],
                             start=first, stop=last, skip_group_check=True)
            nc.tensor.matmul(ACC[2 + h][64:128, :], SELP, w_t[:, hs],
                             start=first, stop=last, skip_group_check=True)

    # ---- out = acc / wsum ----
    # wsum lives at partitions 64:128 of ACC[2+h]; DMA-replicate it to both
    # partition halves so every elementwise op sees matching base partitions.
    for h in range(2):
        hs = slice(h * 512, h * 512 + 512)
        AS = opool.tile([128, 512], BF16, tag="as", name=f"as_{mi}_{h}")
        nc.scalar.copy(AS[:], ACC[2 + h][:])
        TMP = ppool.tile([128, 512], FP32, tag="SP", name=f"wtmp_{mi}_{h}",
                         bufs=4)
        nc.tensor.matmul(TMP[0:64, :], SHIFTC, AS[:], start=True, stop=True,
                         skip_group_check=True)
        RW = opool.tile([128, 512], FP32, tag="rw", name=f"rw_{mi}_{h}")
        nc.vector.reciprocal(RW[0:64, :], TMP[0:64, :])
        nc.vector.reciprocal(RW[64:128, :], ACC[2 + h][64:128, :])
        o = opool.tile([128, 512], FP32, tag="o", name=f"o_{mi}_{h}")
        o2 = opool.tile([128, 512], FP32, tag="o2", name=f"o2_{mi}_{h}")
        nc.vector.tensor_mul(o[0:64, :], ACC[h][0:64, :], RW[0:64, :])
        nc.vector.tensor_mul(o[64:128, :], ACC[h][64:128, :], RW[64:128, :])
        nc.vector.tensor_mul(o2[0:64, :], ACC[2 + h][0:64, :], RW[0:64, :])
        nc.sync.dma_start(out=y[0, r0:r0 + MR, hs], in_=o[0:64, :])
        nc.sync.dma_start(out=y[1, r0:r0 + MR, hs], in_=o[64:128, :])
        nc.sync.dma_start(out=y[2, r0:r0 + MR, hs], in_=o2[0:64, :])


def shard_inputs(input):
    """input [2,18,1024,1024] f32 -> per-core {fil, prm, cst} bf16."""
    input = np.asarray(input, dtype=np.float32)
    cst = build_consts()
    fil_all = input[:, :CF]
    prm_all = input[:, CF:]
    in_maps = []
    for core in range(8):
        b, q = divmod(core, 4)
        r0 = q * HC
        slab = np.full((CF, HIN, WP), SENT, dtype=np.float32)
        s_lo = max(r0 - RAD, 0)
        s_hi = min(r0 + HC + RAD, H)
        slab[:, s_lo - (r0 - RAD):s_hi - (r0 - RAD), RAD:RAD + W] = \
            fil_all[b, :, s_lo:s_hi, :]
        prm = prm_all[b, :, r0:r0 + HC, :]
        in_maps.append({
            "fil": np.ascontiguousarray(slab.astype(BF)),
            "prm": np.ascontiguousarray(prm.astype(BF)),
            "cst": cst,
        })
    return in_maps


def assemble(results):
    out = np.empty((B, CO, H, W), dtype=np.float32)
    for core in range(8):
        b, q = divmod(core, 4)
        out[b, :, q * HC:(q + 1) * HC, :] = results[core]["y"]
    return out


def kernel(input):
    from concourse.bass_utils import run_bass_kernel_spmd

    if "nc" not in _CACHED:
        _CACHED["nc"] = build_nc()
    in_maps = shard_inputs(input)
    res = run_bass_kernel_spmd(_CACHED["nc"], in_maps, list(range(8)))
    return assemble(res.results)
